# revision 71
# baseline (speedup 1.0000x reference)
"""Trainium2 Bass kernel for the delayed-dense spiking network.

Network (reference semantics):
    s1 = spike(delayed_dense(psp(x),  w1, d1))   # [B, 800, T]
    s3 = spike(delayed_dense(psp(s1), w3, d3))   # [B, 10, T]

psp is a linear causal filter (u[t] = a*u[t-1] + s[t]) and delayed_dense is a
shift-grouped GEMM; psp commutes exactly with the time shifts and (up to fp
rounding ~1e-6 rel) with the channel mixing, so each layer is evaluated as
    spike(psp(sum_s W_s @ shift_s(x)))
which lets both GEMMs run on *binary* activations (exact in fp8).  Delays lie
in [0,4) so only shifts 0..4 are live (5 shift matrices).

Sharding: data-parallel over batch, 8 batch elements per NeuronCore.

Engine assignment (the v1 bottleneck was the DVE):
  PE:   L1 GEMM (all 5 shifts packed along K in fp8 DoubleRow, 32 subtiles ->
        16 uniform DR pairs) + L2 GEMM (shift-stacked M=50, fp8 DR, k-pairs
        interleaved incrementally under the L1 m-loop as s1 subtiles land) +
        float32r selector matmuls for the tail merge.
  DVE:  psp scans (PSUM f32 -> f32, 3 chunks with state handoff), spike
        thresholds (2D APs only -- 3D/4D APs measured ~4x slower on real
        hardware), L2 merge bf16 adds (2x_1p fast path), L2 psp scans.
  ACT:  zero-padding memzeros (emitted after all prologue DMA issues so the
        queue stays free to issue xg chunks), one tail PSUM copy.
  Pool: nothing (GPSIMD ops measured ~2us each on real hardware; the cost
        model prices them ~4x cheaper).
  SP/ACT: DMA issue.

Batches are packed 4 per group along the matmul free axis (2 groups) and
every matmul streams >=384 columns (chunks 512/512/384): on hardware each
matmul's LDWEIGHTS (~213ns, not modeled by the cost model) only hides under
a >=~512-column stream, so thin matmuls are weight-load-bound.  PSUM: L1
accumulators are 1-bank [128,512] tiles (6 = double-buffered 3-chunk
chains); psp scans chain across chunk tiles via initial=prev[:, -1:] state
handoff in f32 (exact).  L2 runs in two sequential half-group passes
(slots 0,1 then 2,3) through one 2-bank [64,1024] accumulator whose batch
slots live in separate banks.

Host-side prep:
  - w1f:  all-shift masked transposed weights, m-tile-major contiguous:
          [128, 6, 31, 128] fp8e4m3 (+ w1f6 [128, 31, 32] for the 32-row
          7th m-tile); the all-zero 32nd k-subtile is zeroed on-chip.
  - w3t:  shift-stacked transposed weights: [128, 512] fp8e4m3
          (SBUF view [128, 8, 64]; col = s*10 + o, zero-padded).
  - xg:   shift-replicated binary input, 4 batches packed along the free
          axis: [2, 3968, 1408] fp8e4m3.
  - sel:  shift-unstack selector, sel[s*10+o, s, o] = 1: [50, 5, 10] f32.
"""

import numpy as np
import ml_dtypes

NIN, NHID, NOUT = 784, 800, 10
B, T = 64, 350
NSHIFT = 5            # delays in [0,4) touch integer shifts 0..4
TAU = 10.0
THETA = 10.0
DMAX = 4.0
N_CORES = 8
BPC = B // N_CORES    # batches per core
GB = 4                # batches per group (packed along matmul free axis)
NG = BPC // GB        # groups per core
KF_TILES = 32         # all 5 shifts in fp8: ceil(5*784/128)=31, padded even
KF_PAD = KF_TILES * 128   # 4096
TF = 352              # per-batch xg slot width (DoubleRow offsets % 16)
GW = GB * TF          # 1408: group width (4 batches side by side)
CHUNKS = ((0, 512), (512, 1024), (1024, 1408))   # PSUM-bank-sized N chunks
M1_TILES = 7          # ceil(800/128)
TW2 = 368             # s1 slot width (subtile step % 16)
SW2 = GB * TW2        # 1472
K2P_TILES = 8         # layer-2 contraction padded to 8 subtiles (4 DR pairs)
M2 = NSHIFT * NOUT    # 50
M2P = 64              # layer-2 lhsT free width

DECAY = float(np.float32(np.exp(np.float64(-1.0 / TAU))))


def _masked_shift_weights(w, d):
    """Return list of NSHIFT float32 [O, I] shift matrices (linear interp)."""
    d = np.clip(d.astype(np.float32), 0.0, np.float32(DMAX))
    fl = np.floor(d)
    frac = d - fl
    out = []
    for s in range(NSHIFT):
        ws = w * ((fl == s).astype(np.float32) * (1.0 - frac)
                  + (fl == (s - 1)).astype(np.float32) * frac)
        out.append(ws.astype(np.float32))
    return out


def _prep_host(spike_input, w1, d1, w3, d3):
    f8 = ml_dtypes.float8_e4m3
    w1s = _masked_shift_weights(w1, d1)           # 5 x [800, 784]
    w1t = np.zeros((KF_PAD, NHID), dtype=f8)      # [K, M]
    for s in range(NSHIFT):
        w1t[s * NIN:(s + 1) * NIN, :] = w1s[s].T.astype(f8)
    # m-tile-major contiguous layouts; the all-zero 32nd k-subtile is not
    # shipped (SBUF-side memset instead)
    w1f = np.zeros((128, 6, KF_TILES - 1, 128), dtype=f8)
    for mt in range(6):
        for k in range(KF_TILES - 1):
            w1f[:, mt, k, :] = w1t[k * 128:(k + 1) * 128, mt * 128:(mt + 1) * 128]
    w1f6 = np.zeros((128, KF_TILES - 1, 32), dtype=f8)
    for k in range(KF_TILES - 1):
        w1f6[:, k, :] = w1t[k * 128:(k + 1) * 128, 768:800]

    w3s = _masked_shift_weights(w3, d3)           # 5 x [10, 800]
    w3tt = np.zeros((K2P_TILES * 128, M2P), dtype=f8)
    for s in range(NSHIFT):
        w3tt[:NHID, s * NOUT:(s + 1) * NOUT] = w3s[s].T.astype(f8)
    w3t = np.zeros((128, K2P_TILES * M2P), dtype=f8)
    for k in range(K2P_TILES):
        w3t[:, k * M2P:(k + 1) * M2P] = w3tt[k * 128:(k + 1) * 128, :]

    xf8 = spike_input.astype(f8)                  # binary -> exact in fp8
    # xg[g][s*NIN + i, b*TF + t] = x[GB*g + b, i, t - s]; the zero 32nd
    # k-subtile (rows 3968+) is not shipped
    xg = np.zeros((B // GB, KF_PAD, GW), dtype=f8)
    for g in range(B // GB):
        for b in range(GB):
            xb = xf8[g * GB + b]
            for s in range(NSHIFT):
                xg[g, s * NIN:(s + 1) * NIN, b * TF + s:b * TF + T] = \
                    xb[:, :T - s]

    sel = np.zeros((M2, NSHIFT, NOUT), dtype=np.float32)
    for s in range(NSHIFT):
        for o in range(NOUT):
            sel[s * NOUT + o, s, o] = 1.0
    return xg[:, :31 * 128, :], w1f, w1f6, w3t, sel


def _build_nc(n_batch=BPC, rep=1):
    import contextlib
    import concourse.bacc as bacc
    import concourse.mybir as mybir
    import concourse.tile as tile

    f32 = mybir.dt.float32
    f8 = mybir.dt.float8e4

    nc = bacc.Bacc(None, target_bir_lowering=False, debug=False)
    xg_d = nc.dram_tensor("xg", [NG, KF_PAD - 128, GW], f8, kind="ExternalInput")
    w1f_d = nc.dram_tensor("w1f", [128, 6, KF_TILES - 1, 128], f8, kind="ExternalInput")
    w1f6_d = nc.dram_tensor("w1f6", [128, KF_TILES - 1, 32], f8, kind="ExternalInput")
    w3t_d = nc.dram_tensor("w3t", [128, K2P_TILES * M2P], f8, kind="ExternalInput")
    sel_d = nc.dram_tensor("sel", [M2, NSHIFT, NOUT], f32, kind="ExternalInput")
    out_d = nc.dram_tensor("out", [n_batch, NOUT, T], f32, kind="ExternalOutput")

    with tile.TileContext(nc) as tc:
        with (
            tc.tile_pool(name="const", bufs=1) as constp,
            tc.tile_pool(name="xpool", bufs=2) as xpool,
            tc.tile_pool(name="upool", bufs=3) as upool,
            tc.tile_pool(name="qpool", bufs=2) as qpool,
            tc.tile_pool(name="opool", bufs=2) as opool,
            tc.tile_pool(name="psum1", bufs=6, space="PSUM") as psum1,
            tc.tile_pool(name="psum2", bufs=1, space="PSUM") as psum2,
        ):
            w1f = constp.tile([128, 6, KF_TILES, 128], f8)
            w1f6 = constp.tile([128, KF_TILES, 32], f8)
            w3t = constp.tile([128, K2P_TILES, M2P], f8)
            dec = constp.tile([128, GW], f32)
            dec2 = constp.tile([NOUT, T], f32)
            sel_f = constp.tile([M2, NSHIFT, NOUT], f32)
            thneg = constp.tile([128, 1], f32)
            sel_r = constp.tile([M2, NSHIFT, NOUT], mybir.dt.float32r)
            s1g = [constp.tile([128, K2P_TILES, SW2], f8, name=f"s1_{g}")
                   for g in range(NG)]

            loop_ctx = (
                tc.For_i(0, rep, 1, hint_engines=(mybir.EngineType.PE,))
                if rep > 1 else contextlib.nullcontext()
            )
            with loop_ctx:
                _emit_body(nc, tc, n_batch, xpool, upool, qpool, opool,
                           psum1, psum2, xg_d, out_d,
                           w1f, w1f_d, w1f6, w1f6_d, w3t, w3t_d,
                           dec, dec2, sel_f, sel_d, sel_r, thneg, s1g, mybir)

    nc.compile()
    return nc


def _emit_body(nc, tc, n_batch, xpool, upool, qpool, opool,
               psum1, psum2, xg_d, out_d, w1f, w1f_d, w1f6, w1f6_d,
               w3t, w3t_d, dec, dec2, sel_f, sel_d, sel_r, thneg, s1g, mybir):
    f32 = mybir.dt.float32
    bf16 = mybir.dt.bfloat16
    f32r = mybir.dt.float32r
    f8 = mybir.dt.float8e4
    mult, add = mybir.AluOpType.mult, mybir.AluOpType.add
    is_ge = mybir.AluOpType.is_ge
    DR = mybir.MatmulPerfMode.DoubleRow

    # ---- consts ----
    nc.vector.memset(dec[:], DECAY)
    for b in range(1, GB):
        nc.vector.memset(dec[:, b * TF:b * TF + 1], 0.0)  # psp reset at seams
    nc.vector.memset(dec2[:], DECAY)
    nc.vector.memset(thneg[:], -THETA)
    # zero 32nd k-subtile of the weight tiles (not shipped over DMA); on the
    # DVE, which is idle during the prologue -- the ACT queue must stay free
    # to issue the odd xg chunks.  (GPSIMD is ~2us/op on real hardware, so
    # nothing compute goes to Pool.)
    nc.vector.memset(w1f[:, :, KF_TILES - 1, :], 0.0)
    nc.vector.memset(w1f6[:, KF_TILES - 1, :], 0.0)

    def emit_s1_padding():
        # s1 padding zeros (the data region [.:mw, m, slot*368+4 : +350] is
        # fully rewritten by the thresholds every iteration; only the padding
        # has to be zero for the L2 DoubleRow reads).  ACT engine, emitted
        # after all prologue DMA issues so they don't block the queue; the
        # padding is disjoint from the threshold writes, so only the L2
        # pairs (first at ~slot 5) depend on them.
        for g in range(NG):
            nc.scalar.memzero(s1g[g][:, 7, :])            # zero 8th subtile
            nc.scalar.memzero(s1g[g][32:64, 6, :])        # m6 tail partitions
            nc.scalar.memzero(s1g[g][64:, 6, :])
            s6 = s1g[g][:32, 6, :].rearrange("p (b t) -> p b t", t=TW2)
            nc.scalar.memzero(s6[:, :, 0:4])
            nc.vector.memset(s6[:, :, 4 + T:TW2], 0.0)   # 14-wide: ACT %4
            sk = s1g[g][:, 0:6, :].rearrange("p k (b t) -> p k b t", t=TW2)
            nc.scalar.memzero(sk[:, :, :, 0:4])
            nc.vector.memset(sk[:, :, :, 4 + T:TW2], 0.0)

    # ---- DMA issue ----
    # Group 0 is DMA-bound: every L1 chain reads all xg subtiles, so the
    # first two chains run k-major (2 concurrent 3-tile accumulators) and
    # consume chunks as they land.  Transfers serialize on the shared DMA
    # engines in rough issue order, so streams are laid out by need-time:
    # w1f m0/m1 half-slices first, then the 16 xg0 chunks with w1f m2
    # slipped in near the end, then w1f m3-m6 and the xg1 parts.
    xg0 = xpool.tile([128, KF_TILES, GW], f8, tag="xg", name="xg_0")
    xg1 = xpool.tile([128, KF_TILES, GW], f8, tag="xg", name="xg_1")
    xgt = [xg0, xg1]

    def xg_chunk(eng, g, c):
        src = xg_d[g].rearrange("(k p) c -> p k c", p=128)
        k0, k1 = 2 * c, min(2 * c + 2, KF_TILES - 1)
        eng.dma_start(xgt[g][:, k0:k1, :], src[:, k0:k1, :])

    nc.vector.memset(xg0[:, KF_TILES - 1, :], 0.0)
    nc.vector.memset(xg1[:, KF_TILES - 1, :], 0.0)
    nc.sync.dma_start(w1f[:, 0, 0:16, :], w1f_d[:, 0, 0:16])
    nc.scalar.dma_start(w1f[:, 1, 0:16, :], w1f_d[:, 1, 0:16])
    xg_chunk(nc.sync, 0, 0)
    nc.sync.dma_start(w1f[:, 0, 16:31, :], w1f_d[:, 0, 16:31])
    xg_chunk(nc.scalar, 0, 1)
    nc.scalar.dma_start(w1f[:, 1, 16:31, :], w1f_d[:, 1, 16:31])
    for c in range(2, 13):
        xg_chunk(nc.sync if c % 2 == 0 else nc.scalar, 0, c)
    nc.scalar.dma_start(w1f[:, 2, 0:31, :], w1f_d[:, 2])
    xg_chunk(nc.sync, 0, 13)
    xg_chunk(nc.sync, 0, 14)
    xg_chunk(nc.scalar, 0, 15)
    nc.sync.dma_start(w1f[:, 3, 0:31, :], w1f_d[:, 3])
    nc.scalar.dma_start(w1f[:, 4, 0:31, :], w1f_d[:, 4])
    nc.sync.dma_start(w1f[:, 5, 0:31, :], w1f_d[:, 5])
    nc.scalar.dma_start(w1f6[:, 0:31, :], w1f6_d[:])
    nc.scalar.dma_start(w3t[:], w3t_d.rearrange("p (k c) -> p k c", c=M2P))
    nc.scalar.dma_start(sel_f[:], sel_d[:])
    nc.vector.tensor_copy(sel_r[:], sel_f[:])
    # xg1 lands fully under g0's m-loop
    for q in range(4):
        eng = nc.sync if q % 2 == 0 else nc.scalar
        k0, k1 = 8 * q, min(8 * q + 8, KF_TILES - 1)
        src = xg_d[1].rearrange("(k p) c -> p k c", p=128)
        eng.dma_start(xg1[:, k0:k1, :], src[:, k0:k1, :])
    emit_s1_padding()

    # ---- L2 helpers ----
    # L2 runs in two sequential half-group passes (slots 0,1 then 2,3); each
    # half accumulates its two batch slots in separate PSUM banks (cols 0:368
    # and 512:880 of a [64,1024] tile) so reads never wait on the other slot.
    p3h = {}

    def l2_tile(g, h, from_psum1=False):
        # returns [(tile, col_base)] per local slot
        key = (g, h)
        if key not in p3h:
            if from_psum1:
                p3h[key] = [
                    (psum1.tile([M2P, TW2], f32, tag="ph",
                                name=f"p3_{g}_{h}_{sl}"), 0)
                    for sl in range(2)]
            else:
                t = psum2.tile([M2P, 1024], f32, tag="p3", name=f"p3_{g}_{h}")
                p3h[key] = [(t, 0), (t, 512)]
        return p3h[key]

    def emit_l2_pair(g, h, p, from_psum1=False):
        tiles = l2_tile(g, h, from_psum1)
        for sl in range(2):
            b = 2 * h + sl
            t, c0 = tiles[sl]
            nc.tensor.matmul(
                t[:, c0:c0 + TW2],
                w3t[:, 2 * p:2 * p + 2, :],
                s1g[g][:, 2 * p:2 * p + 2, b * TW2:(b + 1) * TW2],
                start=(p == 0), stop=(p == 3),
                perf_mode=DR,
            )

    def emit_l2_merge(g, h):
        # regroup the 50 stacked shift rows into a [10, 5, .] tile with 5
        # tiny SBUF->SBUF DMAs (SBUF-op APs must start at partition
        # 0/32/64/96), then merge with bf16 adds (DVE 2x_1p fast path).
        tiles = l2_tile(g, h)
        q50 = qpool.tile([M2, 2, TW2], bf16, tag="q50", name=f"q50_{g}_{h}")
        for sl in range(2):
            t, c0 = tiles[sl]
            nc.vector.tensor_copy(q50[:, sl, :], t[:M2, c0:c0 + TW2])
        q5 = qpool.tile([NOUT, NSHIFT, 2, TW2], bf16, tag="q5", name=f"q5_{g}_{h}")
        dma_eng = [nc.scalar, nc.sync, nc.scalar, nc.sync, nc.scalar]
        for s in range(NSHIFT):
            dma_eng[s].dma_start(q5[:, s, :, :], q50[s * NOUT:(s + 1) * NOUT, :, :])
        acc = opool.tile([NOUT, 2 * T], bf16, tag="acc", name=f"acc_{g}_{h}")
        for sl in range(2):
            a = acc[:, sl * T:(sl + 1) * T]
            nc.vector.tensor_add(a, q5[:, 0, sl, 4:4 + T], q5[:, 1, sl, 3:3 + T])
            nc.vector.tensor_add(a, a, q5[:, 2, sl, 2:2 + T])
            nc.vector.tensor_add(a, a, q5[:, 3, sl, 1:1 + T])
            nc.vector.tensor_add(a, a, q5[:, 4, sl, 0:0 + T])
        return acc

    def emit_l2_out(g, h, acc):
        u3 = opool.tile([NOUT, 2 * T], bf16, tag="u3", name=f"u3_{g}_{h}")
        o3 = opool.tile([NOUT, 2 * T], f32, tag="o3", name=f"o3_{g}_{h}")
        for sl in range(2):
            nc.vector.tensor_tensor_scan(
                u3[:, sl * T:(sl + 1) * T], dec2[:],
                acc[:, sl * T:(sl + 1) * T], 0.0, mult, add)
        for sl in range(2):
            nc.vector.tensor_scalar(
                out=o3[:, sl * T:(sl + 1) * T], in0=u3[:, sl * T:(sl + 1) * T],
                scalar1=THETA, scalar2=None, op0=is_ge)
        for sl in range(2):
            nc.sync.dma_start(out_d[GB * g + 2 * h + sl],
                              o3[:, sl * T:(sl + 1) * T])

    def emit_l2_tail_half(g, h, from_psum1):
        # selector-matmul merge on the (idle) PE, one chain per batch slot:
        # copy -> 5 float32r selector matmuls -> psp scan -> threshold -> DMA.
        # Copies alternate DVE/ACT so the two slot chains run in parallel.
        tiles = l2_tile(g, h, from_psum1)
        u3 = opool.tile([NOUT, 2 * T], bf16, tag="u3", name=f"u3_{g}_{h}")
        o3 = opool.tile([NOUT, 2 * T], f32, tag="o3", name=f"o3_{g}_{h}")
        for sl in range(2):
            t, c0 = tiles[sl]
            q50r = qpool.tile([M2, TW2], f32r, tag=f"q50r{sl}",
                              name=f"q50r_{g}_{h}_{sl}")
            if sl == 0:
                nc.vector.tensor_copy(q50r[:], t[:M2, c0:c0 + TW2])
            else:
                nc.scalar.copy(q50r[:], t[:M2, c0:c0 + TW2])
            h3p = psum1.tile([128, 512], f32, tag="ph", name=f"h3p_{g}_{h}_{sl}")
            for s in range(NSHIFT):
                nc.tensor.matmul(
                    h3p[:NOUT, 0:T], sel_r[:, s, :],
                    q50r[:, 4 - s:4 - s + T],
                    start=(s == 0), stop=(s == NSHIFT - 1),
                )
            nc.vector.tensor_tensor_scan(
                u3[:, sl * T:(sl + 1) * T], dec2[:],
                h3p[:NOUT, 0:T], 0.0, mult, add)
            nc.vector.tensor_scalar(
                out=o3[:, sl * T:(sl + 1) * T], in0=u3[:, sl * T:(sl + 1) * T],
                scalar1=THETA, scalar2=None, op0=is_ge)
            nc.sync.dma_start(out_d[GB * g + 2 * h + sl],
                              o3[:, sl * T:(sl + 1) * T])

    # deferred L2 work: items run right before chain (g, m); overflow -> tail
    sched = [[[] for _ in range(M1_TILES)] for _ in range(NG)]
    tail = []

    def defer(g, m, item):
        if m < M1_TILES:
            sched[g][m].append(item)
        elif g + 1 < NG:
            sched[g + 1][m - M1_TILES].append(item)
        else:
            tail.append(item)

    accs = {}

    def run_item(item):
        kind, g, h, arg = item
        if kind == "pair":
            emit_l2_pair(g, h, arg)
        elif kind == "merge":
            accs[(g, h)] = emit_l2_merge(g, h)
        else:
            emit_l2_out(g, h, accs[(g, h)])

    for g in range(NG):
        defer(g, 5, ("pair", g, 0, 0))
        defer(g, 6, ("pair", g, 0, 1))
        if g < NG - 1:
            defer(g, 8, ("pair", g, 0, 2))
            defer(g, 9, ("pair", g, 0, 3))
            defer(g, 9, ("merge", g, 0, None))
            defer(g, 10, ("pair", g, 1, 0))
            defer(g, 10, ("pair", g, 1, 1))
            defer(g, 10, ("out", g, 0, None))
            defer(g, 10, ("pair", g, 1, 2))
            defer(g, 10, ("pair", g, 1, 3))
            defer(g, 11, ("merge", g, 1, None))
            defer(g, 12, ("out", g, 1, None))

    def emit_post(g, m, phs):
        # psp scans per PSUM chunk tile, chained via f32 state handoff (the
        # scan state is f32 internally and u is f32, so this is bit-identical
        # to one long scan); dec is zeroed at slot seams.  One merged 3D-AP
        # threshold covers all 4 batch slots; the very last m-tile keeps
        # per-slot thresholds so the tail's L2 chains start earlier.
        mw = 128 if m < 6 else 32
        u = upool.tile([128, GW], f32, tag="u", name=f"u_{g}_{m}")
        for ci, (c0, c1) in enumerate(CHUNKS):
            nc.vector.tensor_tensor_scan(
                u[:mw, c0:c1], dec[:mw, c0:c1], phs[ci][:mw, 0:c1 - c0],
                0.0 if ci == 0 else u[:mw, c0 - 1:c0], mult, add)
            for b in range(GB):
                if c1 >= b * TF + T > (c0 if ci else 0):
                    if g == NG - 1 and m in (4, 5) and b >= 2:
                        # relu(sign(u-theta)) on ACT == (u >= theta) except
                        # at exact ties; keeps the DVE clear right before
                        # the tail, whose gating thresholds live there
                        sg = opool.tile([128, T], bf16, tag="sgn",
                                        name=f"sgn_{m}_{b}", bufs=2)
                        nc.scalar.activation(
                            sg[:mw, :], u[:mw, b * TF:b * TF + T],
                            mybir.ActivationFunctionType.Sign,
                            bias=thneg[:mw])
                        nc.scalar.activation(
                            s1g[g][:mw, m, b * TW2 + 4:b * TW2 + 4 + T],
                            sg[:mw, :],
                            mybir.ActivationFunctionType.Relu)
                    else:
                        nc.vector.tensor_scalar(
                            out=s1g[g][:mw, m, b * TW2 + 4:b * TW2 + 4 + T],
                            in0=u[:mw, b * TF:b * TF + T],
                            scalar1=THETA, scalar2=None, op0=is_ge)

    def chain_tiles(g, m):
        return [psum1.tile([128, c1 - c0], f32, tag="ph",
                           name=f"ph_{g}_{m}_{ci}")
                for ci, (c0, c1) in enumerate(CHUNKS)]

    def emit_chain_pair(g, m, phs, j):
        lhs = w1f[:, m, 2 * j:2 * j + 2, :] if m < 6 else \
            w1f6[:, 2 * j:2 * j + 2, :]
        mw = 128 if m < 6 else 32
        for ci, (c0, c1) in enumerate(CHUNKS):
            nc.tensor.matmul(
                phs[ci][:mw, 0:c1 - c0], lhs, xgt[g][:, 2 * j:2 * j + 2, c0:c1],
                start=(j == 0), stop=(j == KF_TILES // 2 - 1),
                perf_mode=DR,
            )

    def emit_chain(g, m, phs):
        # pair-major: the three chunk matmuls of a pair share one lhsT, and
        # consecutive same-lhsT matmuls skip the ~213ns weight reload on
        # hardware (chunk-major ordering measured 1.6x slower end-to-end).
        # The very last chain runs chunks A+B for all pairs first, then C:
        # its A/B scans (and the tail's slot-0/1 chains) start ~1.5us
        # earlier, for one chain's worth of exposed C-chunk weight loads.
        if g == NG - 1 and m >= M1_TILES - 2:
            lhs = (lambda j: w1f[:, m, 2 * j:2 * j + 2, :]) if m < 6 else \
                (lambda j: w1f6[:, 2 * j:2 * j + 2, :])
            mw = 128 if m < 6 else 32
            for j in range(KF_TILES // 2):
                for ci in (0, 1):
                    c0, c1 = CHUNKS[ci]
                    nc.tensor.matmul(
                        phs[ci][:mw, 0:c1 - c0], lhs(j),
                        xgt[g][:, 2 * j:2 * j + 2, c0:c1],
                        start=(j == 0), stop=(j == KF_TILES // 2 - 1),
                        perf_mode=DR,
                    )
                if j == 7 and m == M1_TILES - 1:
                    # inject the L2 pairs whose threshold deps (m0-m3) are
                    # long resolved; the pairs needing m4/m5 thresholds go
                    # at the end of the C pass (the DVE clears m5's
                    # postamble roughly then)
                    emit_l2_pair(g, 1, 0, from_psum1=True)
                    emit_l2_pair(g, 1, 1, from_psum1=True)
            c0, c1 = CHUNKS[2]
            for j in range(KF_TILES // 2):
                nc.tensor.matmul(
                    phs[2][:mw, 0:c1 - c0], lhs(j),
                    xgt[g][:, 2 * j:2 * j + 2, c0:c1],
                    start=(j == 0), stop=(j == KF_TILES // 2 - 1),
                    perf_mode=DR,
                )
                if j == 13 and m == M1_TILES - 1:
                    emit_l2_pair(g, 0, 2)
                    emit_l2_pair(g, 1, 2, from_psum1=True)
        else:
            for j in range(KF_TILES // 2):
                emit_chain_pair(g, m, phs, j)

    for g in range(NG):
        if g == 0:
            # k-major over m0/m1 plus m2's first two chunks: the two 3-tile
            # accumulators live in psum1, m2's A+B chunks borrow the (still
            # idle) 2-bank psum2 ring buffer, so 2.7 chains consume each xg0
            # chunk as it lands instead of serializing behind the load.
            for m in range(3):
                for item in sched[g][m]:
                    run_item(item)
            kphs = [chain_tiles(0, m) for m in range(2)]
            m2ab = psum2.tile([128, 1024], f32, tag="p3", name="ph_0_2ab")
            for j in range(KF_TILES // 2):
                for m in range(2):
                    emit_chain_pair(0, m, kphs[m], j)
                for (c0, c1) in ((0, 512), (512, 1024)):
                    nc.tensor.matmul(
                        m2ab[:, c0:c1], w1f[:, 2, 2 * j:2 * j + 2, :],
                        xgt[0][:, 2 * j:2 * j + 2, c0:c1],
                        start=(j == 0), stop=(j == KF_TILES // 2 - 1),
                        perf_mode=DR,
                    )
            for m in range(2):
                emit_post(0, m, kphs[m])
            # m2: chunk C chain (data resident by now) + scans
            phC = psum1.tile([128, 384], f32, tag="ph", name="ph_0_2_2")
            for j in range(KF_TILES // 2):
                nc.tensor.matmul(
                    phC[:, 0:384], w1f[:, 2, 2 * j:2 * j + 2, :],
                    xgt[0][:, 2 * j:2 * j + 2, 1024:1408],
                    start=(j == 0), stop=(j == KF_TILES // 2 - 1),
                    perf_mode=DR,
                )
            u = upool.tile([128, GW], f32, tag="u", name="u_0_2")
            nc.vector.tensor_tensor_scan(
                u[:, 0:512], dec[:, 0:512], m2ab[:, 0:512], 0.0, mult, add)
            nc.vector.tensor_tensor_scan(
                u[:, 512:1024], dec[:, 512:1024], m2ab[:, 512:1024],
                u[:, 511:512], mult, add)
            nc.vector.tensor_tensor_scan(
                u[:, 1024:1408], dec[:, 1024:1408], phC[:, 0:384],
                u[:, 1023:1024], mult, add)
            for b in range(GB):
                nc.vector.tensor_scalar(
                    out=s1g[0][:, 2, b * TW2 + 4:b * TW2 + 4 + T],
                    in0=u[:, b * TF:b * TF + T],
                    scalar1=THETA, scalar2=None, op0=is_ge)
            m_range = range(3, M1_TILES)
        else:
            m_range = range(M1_TILES)
        for m in m_range:
            for item in sched[g][m]:
                run_item(item)
            phs = chain_tiles(g, m)
            emit_chain(g, m, phs)
            emit_post(g, m, phs)

    # ---- tail: last group's L2 ----
    # Four per-slot selector chains, interleaved so every engine stream is
    # in dependency order: PE [H0p3, mms0, H1p3, mms1, mms2, mms3],
    # DVE [copy1, scan0, thr0, scan1, thr1, copy3, scan2, thr2, scan3,
    # thr3], ACT [copy0, copy2].  H0p2/H1p0-2 were injected mid-chain.
    g = NG - 1
    for item in tail:
        run_item(item)
    tiles0 = l2_tile(g, 0)
    tiles1 = l2_tile(g, 1, True)
    u3 = opool.tile([NOUT, GB * T], bf16, tag="u3t", name="u3_tail", bufs=1)
    o3 = opool.tile([NOUT, GB * T], f32, tag="o3t", name="o3_tail", bufs=1)

    def tail_copy(sl):
        t, c0 = (tiles0 if sl < 2 else tiles1)[sl % 2]
        q = qpool.tile([M2, TW2], f32r, tag=f"q50r{sl}", name=f"q50rt_{sl}",
                       bufs=1)
        nc.scalar.copy(q[:], t[:M2, c0:c0 + TW2])
        return q

    def tail_mms(sl, q):
        h3p = psum1.tile([128, 512], f32, tag="ph", name=f"h3pt_{sl}")
        for s in range(NSHIFT):
            nc.tensor.matmul(
                h3p[:NOUT, 0:T], sel_r[:, s, :], q[:, 4 - s:4 - s + T],
                start=(s == 0), stop=(s == NSHIFT - 1))
        return h3p

    def tail_fin(sl, h3p, use_act=False):
        nc.vector.tensor_tensor_scan(
            u3[:, sl * T:(sl + 1) * T], dec2[:], h3p[:NOUT, 0:T],
            0.0, mult, add)
        if use_act:
            # relu(sign(u - theta)) == (u >= theta) except at u == theta
            # exactly; u3 tops out ~3.6 here, so the tie case is unreachable.
            tmp = opool.tile([NOUT, T], f32, tag="sgn", name=f"sgn_{sl}",
                             bufs=1)
            nc.scalar.activation(tmp[:], u3[:, sl * T:(sl + 1) * T],
                                 mybir.ActivationFunctionType.Sign,
                                 bias=thneg[:NOUT])
            nc.scalar.activation(o3[:, sl * T:(sl + 1) * T], tmp[:],
                                 mybir.ActivationFunctionType.Relu)
        else:
            nc.vector.tensor_scalar(
                out=o3[:, sl * T:(sl + 1) * T],
                in0=u3[:, sl * T:(sl + 1) * T],
                scalar1=THETA, scalar2=None, op0=is_ge)
        nc.sync.dma_start(out_d[GB * g + sl], o3[:, sl * T:(sl + 1) * T])

    emit_l2_pair(g, 0, 3)
    q0 = tail_copy(0)
    q1 = tail_copy(1)
    h0 = tail_mms(0, q0)
    emit_l2_pair(g, 1, 3, from_psum1=True)
    h1 = tail_mms(1, q1)
    tail_fin(0, h0)
    q2 = tail_copy(2)
    h2 = tail_mms(2, q2)
    tail_fin(1, h1)
    q3 = tail_copy(3)
    h3 = tail_mms(3, q3)
    tail_fin(2, h2)
    tail_fin(3, h3)


def make_in_maps(spike_input, w1, d1, w3, d3):
    xg, w1f, w1f6, w3t, sel = _prep_host(spike_input, w1, d1, w3, d3)
    in_maps = []
    for c in range(N_CORES):
        in_maps.append({
            "xg": np.ascontiguousarray(xg[c * NG:(c + 1) * NG]),
            "w1f": w1f,
            "w1f6": w1f6,
            "w3t": w3t,
            "sel": sel,
        })
    return in_maps


def kernel(spike_input, w1, d1, w3, d3):
    from concourse import bass_utils

    spike_input = np.asarray(spike_input, dtype=np.float32)
    w1 = np.asarray(w1, dtype=np.float32)
    d1 = np.asarray(d1, dtype=np.float32)
    w3 = np.asarray(w3, dtype=np.float32)
    d3 = np.asarray(d3, dtype=np.float32)

    nc = _build_nc()
    in_maps = make_in_maps(spike_input, w1, d1, w3, d3)
    res = bass_utils.run_bass_kernel_spmd(nc, in_maps, core_ids=list(range(N_CORES)))
    out = np.concatenate([res.results[c]["out"] for c in range(N_CORES)], axis=0)
    return out.astype(np.float32)


# revision 73
# speedup vs baseline: 1.1291x; 1.1291x over previous
"""Trainium2 Bass kernel for the delayed-dense spiking network.

Network (reference semantics):
    s1 = spike(delayed_dense(psp(x),  w1, d1))   # [B, 800, T]
    s3 = spike(delayed_dense(psp(s1), w3, d3))   # [B, 10, T]

psp is a linear causal filter (u[t] = a*u[t-1] + s[t]) and delayed_dense is a
shift-grouped GEMM; psp commutes exactly with the time shifts and (up to fp
rounding ~1e-6 rel) with the channel mixing, so each layer is evaluated as
    spike(psp(sum_s W_s @ shift_s(x)))
which lets both GEMMs run on *binary* activations (exact in fp8).  Delays lie
in [0,4) so only shifts 0..4 are live (5 shift matrices).

Sharding: data-parallel over batch, 8 batch elements per NeuronCore.

Engine assignment (the v1 bottleneck was the DVE):
  PE:   L1 GEMM (all 5 shifts packed along K in fp8 DoubleRow, 32 subtiles ->
        16 uniform DR pairs) + L2 GEMM (shift-stacked M=50, fp8 DR, k-pairs
        interleaved incrementally under the L1 m-loop as s1 subtiles land) +
        float32r selector matmuls for the tail merge.
  DVE:  psp scans (PSUM f32 -> f32, 3 chunks with state handoff), spike
        thresholds (2D APs only -- 3D/4D APs measured ~4x slower on real
        hardware), L2 merge bf16 adds (2x_1p fast path), L2 psp scans.
  ACT:  zero-padding memzeros (emitted after all prologue DMA issues so the
        queue stays free to issue xg chunks), one tail PSUM copy.
  Pool: nothing (GPSIMD ops measured ~2us each on real hardware; the cost
        model prices them ~4x cheaper).
  SP/ACT: DMA issue.

Batches are packed 4 per group along the matmul free axis (2 groups) and
every matmul streams >=384 columns (chunks 512/512/384): on hardware each
matmul's LDWEIGHTS (~213ns, not modeled by the cost model) only hides under
a >=~512-column stream, so thin matmuls are weight-load-bound.  PSUM: L1
accumulators are 1-bank [128,512] tiles (6 = double-buffered 3-chunk
chains); psp scans chain across chunk tiles via initial=prev[:, -1:] state
handoff in f32 (exact).  L2 runs in two sequential half-group passes
(slots 0,1 then 2,3) through one 2-bank [64,1024] accumulator whose batch
slots live in separate banks.

Host-side prep:
  - w1f:  all-shift masked transposed weights, m-tile-major contiguous:
          [128, 6, 31, 128] fp8e4m3 (+ w1f6 [128, 31, 32] for the 32-row
          7th m-tile); the all-zero 32nd k-subtile is zeroed on-chip.
  - w3t:  shift-stacked transposed weights: [128, 512] fp8e4m3
          (SBUF view [128, 8, 64]; col = s*10 + o, zero-padded).
  - xg:   shift-replicated binary input, 4 batches packed along the free
          axis: [2, 3968, 1408] fp8e4m3.
  - sel:  shift-unstack selector, sel[s*10+o, s, o] = 1: [50, 5, 10] f32.
"""

import numpy as np
import ml_dtypes

NIN, NHID, NOUT = 784, 800, 10
B, T = 64, 350
NSHIFT = 5            # delays in [0,4) touch integer shifts 0..4
TAU = 10.0
THETA = 10.0
DMAX = 4.0
N_CORES = 8
BPC = B // N_CORES    # batches per core
GB = 4                # batches per group (packed along matmul free axis)
NG = BPC // GB        # groups per core
KF_TILES = 32         # all 5 shifts in fp8: ceil(5*784/128)=31, padded even
KF_PAD = KF_TILES * 128   # 4096
TF = 352              # per-batch xg slot width (DoubleRow offsets % 16)
GW = GB * TF          # 1408: group width (4 batches side by side)
CHUNKS = ((0, 512), (512, 1024), (1024, 1408))   # PSUM-bank-sized N chunks
M1_TILES = 7          # ceil(800/128)
TW2 = 368             # s1 slot width (subtile step % 16)
SW2 = GB * TW2        # 1472
K2P_TILES = 8         # layer-2 contraction padded to 8 subtiles (4 DR pairs)
M2 = NSHIFT * NOUT    # 50
M2P = 64              # layer-2 lhsT free width

DECAY = float(np.float32(np.exp(np.float64(-1.0 / TAU))))


def _masked_shift_weights(w, d):
    """Return list of NSHIFT float32 [O, I] shift matrices (linear interp)."""
    d = np.clip(d.astype(np.float32), 0.0, np.float32(DMAX))
    fl = np.floor(d)
    frac = d - fl
    out = []
    for s in range(NSHIFT):
        ws = w * ((fl == s).astype(np.float32) * (1.0 - frac)
                  + (fl == (s - 1)).astype(np.float32) * frac)
        out.append(ws.astype(np.float32))
    return out


def _prep_host(spike_input, w1, d1, w3, d3):
    f8 = ml_dtypes.float8_e4m3
    w1s = _masked_shift_weights(w1, d1)           # 5 x [800, 784]
    w1t = np.zeros((KF_PAD, NHID), dtype=f8)      # [K, M]
    for s in range(NSHIFT):
        w1t[s * NIN:(s + 1) * NIN, :] = w1s[s].T.astype(f8)
    # m-tile-major contiguous layouts; the all-zero 32nd k-subtile is not
    # shipped (SBUF-side memset instead)
    w1f = np.zeros((128, 6, KF_TILES - 1, 128), dtype=f8)
    for mt in range(6):
        for k in range(KF_TILES - 1):
            w1f[:, mt, k, :] = w1t[k * 128:(k + 1) * 128, mt * 128:(mt + 1) * 128]
    w1f6 = np.zeros((128, KF_TILES - 1, 32), dtype=f8)
    for k in range(KF_TILES - 1):
        w1f6[:, k, :] = w1t[k * 128:(k + 1) * 128, 768:800]

    w3s = _masked_shift_weights(w3, d3)           # 5 x [10, 800]
    w3tt = np.zeros((K2P_TILES * 128, M2P), dtype=f8)
    for s in range(NSHIFT):
        w3tt[:NHID, s * NOUT:(s + 1) * NOUT] = w3s[s].T.astype(f8)
    w3t = np.zeros((128, K2P_TILES * M2P), dtype=f8)
    for k in range(K2P_TILES):
        w3t[:, k * M2P:(k + 1) * M2P] = w3tt[k * 128:(k + 1) * 128, :]

    xf8 = spike_input.astype(f8)                  # binary -> exact in fp8
    # xg[g][s*NIN + i, b*TF + t] = x[GB*g + b, i, t - s]; the zero 32nd
    # k-subtile (rows 3968+) is not shipped
    xg = np.zeros((B // GB, KF_PAD, GW), dtype=f8)
    for g in range(B // GB):
        for b in range(GB):
            xb = xf8[g * GB + b]
            for s in range(NSHIFT):
                xg[g, s * NIN:(s + 1) * NIN, b * TF + s:b * TF + T] = \
                    xb[:, :T - s]

    sel = np.zeros((M2, NSHIFT, NOUT), dtype=np.float32)
    for s in range(NSHIFT):
        for o in range(NOUT):
            sel[s * NOUT + o, s, o] = 1.0
    return xg[:, :31 * 128, :], w1f, w1f6, w3t, sel


def _build_nc(n_batch=BPC, rep=1):
    import contextlib
    import concourse.bacc as bacc
    import concourse.mybir as mybir
    import concourse.tile as tile

    f32 = mybir.dt.float32
    f8 = mybir.dt.float8e4

    nc = bacc.Bacc(None, target_bir_lowering=False, debug=False)
    xg_d = nc.dram_tensor("xg", [NG, KF_PAD - 128, GW], f8, kind="ExternalInput")
    w1f_d = nc.dram_tensor("w1f", [128, 6, KF_TILES - 1, 128], f8, kind="ExternalInput")
    w1f6_d = nc.dram_tensor("w1f6", [128, KF_TILES - 1, 32], f8, kind="ExternalInput")
    w3t_d = nc.dram_tensor("w3t", [128, K2P_TILES * M2P], f8, kind="ExternalInput")
    sel_d = nc.dram_tensor("sel", [M2, NSHIFT, NOUT], f32, kind="ExternalInput")
    out_d = nc.dram_tensor("out", [n_batch, NOUT, T], f32, kind="ExternalOutput")

    with tile.TileContext(nc) as tc:
        with (
            tc.tile_pool(name="const", bufs=1) as constp,
            tc.tile_pool(name="xpool", bufs=2) as xpool,
            tc.tile_pool(name="upool", bufs=3) as upool,
            tc.tile_pool(name="qpool", bufs=2) as qpool,
            tc.tile_pool(name="opool", bufs=2) as opool,
            tc.tile_pool(name="psum1", bufs=6, space="PSUM") as psum1,
            tc.tile_pool(name="psum2", bufs=1, space="PSUM") as psum2,
        ):
            w1f = constp.tile([128, 6, KF_TILES, 128], f8)
            w1f6 = constp.tile([128, KF_TILES, 32], f8)
            w3t = constp.tile([128, K2P_TILES, M2P], f8)
            dec = constp.tile([128, GW], f32)
            dec2 = constp.tile([NOUT, T], f32)
            sel_f = constp.tile([M2, NSHIFT, NOUT], f32)
            thneg = constp.tile([128, 1], f32)
            sel_r = constp.tile([M2, NSHIFT, NOUT], mybir.dt.float32r)
            s1g = [constp.tile([128, K2P_TILES, SW2], f8, name=f"s1_{g}")
                   for g in range(NG)]

            loop_ctx = (
                tc.For_i(0, rep, 1, hint_engines=(mybir.EngineType.PE,))
                if rep > 1 else contextlib.nullcontext()
            )
            with loop_ctx:
                _emit_body(nc, tc, n_batch, xpool, upool, qpool, opool,
                           psum1, psum2, xg_d, out_d,
                           w1f, w1f_d, w1f6, w1f6_d, w3t, w3t_d,
                           dec, dec2, sel_f, sel_d, sel_r, thneg, s1g, mybir)

    nc.compile()
    return nc


def _emit_body(nc, tc, n_batch, xpool, upool, qpool, opool,
               psum1, psum2, xg_d, out_d, w1f, w1f_d, w1f6, w1f6_d,
               w3t, w3t_d, dec, dec2, sel_f, sel_d, sel_r, thneg, s1g, mybir):
    f32 = mybir.dt.float32
    bf16 = mybir.dt.bfloat16
    f32r = mybir.dt.float32r
    f8 = mybir.dt.float8e4
    mult, add = mybir.AluOpType.mult, mybir.AluOpType.add
    is_ge = mybir.AluOpType.is_ge
    DR = mybir.MatmulPerfMode.DoubleRow

    # ---- consts ----
    nc.vector.memset(dec[:], DECAY)
    for b in range(1, GB):
        nc.vector.memset(dec[:, b * TF:b * TF + 1], 0.0)  # psp reset at seams
    nc.vector.memset(dec2[:], DECAY)
    nc.vector.memset(thneg[:], -THETA)
    # zero 32nd k-subtile of the weight tiles (not shipped over DMA); on the
    # DVE, which is idle during the prologue -- the ACT queue must stay free
    # to issue the odd xg chunks.  (GPSIMD is ~2us/op on real hardware, so
    # nothing compute goes to Pool.)
    nc.vector.memset(w1f[:, :, KF_TILES - 1, :], 0.0)
    nc.vector.memset(w1f6[:, KF_TILES - 1, :], 0.0)

    def emit_s1_padding():
        # s1 padding zeros (the data region [.:mw, m, slot*368+4 : +350] is
        # fully rewritten by the thresholds every iteration; only the padding
        # has to be zero for the L2 DoubleRow reads).  ACT engine, emitted
        # after all prologue DMA issues so they don't block the queue; the
        # padding is disjoint from the threshold writes, so only the L2
        # pairs (first at ~slot 5) depend on them.
        for g in range(NG):
            nc.scalar.memzero(s1g[g][:, 7, :])            # zero 8th subtile
            nc.scalar.memzero(s1g[g][32:64, 6, :])        # m6 tail partitions
            nc.scalar.memzero(s1g[g][64:, 6, :])
            s6 = s1g[g][:32, 6, :].rearrange("p (b t) -> p b t", t=TW2)
            nc.scalar.memzero(s6[:, :, 0:4])
            nc.vector.memset(s6[:, :, 4 + T:TW2], 0.0)   # 14-wide: ACT %4
            sk = s1g[g][:, 0:6, :].rearrange("p k (b t) -> p k b t", t=TW2)
            nc.scalar.memzero(sk[:, :, :, 0:4])
            nc.vector.memset(sk[:, :, :, 4 + T:TW2], 0.0)

    # ---- DMA issue ----
    # Group 0 is DMA-bound: every L1 chain reads all xg subtiles, so the
    # first two chains run k-major (2 concurrent 3-tile accumulators) and
    # consume chunks as they land.  Transfers serialize on the shared DMA
    # engines in rough issue order, so streams are laid out by need-time:
    # w1f m0/m1 half-slices first, then the 16 xg0 chunks with w1f m2
    # slipped in near the end, then w1f m3-m6 and the xg1 parts.
    xg0 = xpool.tile([128, KF_TILES, GW], f8, tag="xg", name="xg_0")
    xg1 = xpool.tile([128, KF_TILES, GW], f8, tag="xg", name="xg_1")
    xgt = [xg0, xg1]

    def xg_chunk(eng, g, c):
        src = xg_d[g].rearrange("(k p) c -> p k c", p=128)
        k0, k1 = 2 * c, min(2 * c + 2, KF_TILES - 1)
        eng.dma_start(xgt[g][:, k0:k1, :], src[:, k0:k1, :])

    nc.vector.memset(xg0[:, KF_TILES - 1, :], 0.0)
    nc.vector.memset(xg1[:, KF_TILES - 1, :], 0.0)
    nc.sync.dma_start(w1f[:, 0, 0:16, :], w1f_d[:, 0, 0:16])
    nc.scalar.dma_start(w1f[:, 1, 0:16, :], w1f_d[:, 1, 0:16])
    xg_chunk(nc.sync, 0, 0)
    nc.sync.dma_start(w1f[:, 0, 16:31, :], w1f_d[:, 0, 16:31])
    xg_chunk(nc.scalar, 0, 1)
    nc.scalar.dma_start(w1f[:, 1, 16:31, :], w1f_d[:, 1, 16:31])
    for c in range(2, 13):
        xg_chunk(nc.sync if c % 2 == 0 else nc.scalar, 0, c)
    nc.scalar.dma_start(w1f[:, 2, 0:31, :], w1f_d[:, 2])
    xg_chunk(nc.sync, 0, 13)
    xg_chunk(nc.sync, 0, 14)
    xg_chunk(nc.scalar, 0, 15)
    nc.sync.dma_start(w1f[:, 3, 0:31, :], w1f_d[:, 3])
    nc.scalar.dma_start(w1f[:, 4, 0:31, :], w1f_d[:, 4])
    nc.sync.dma_start(w1f[:, 5, 0:31, :], w1f_d[:, 5])
    nc.scalar.dma_start(w1f6[:, 0:31, :], w1f6_d[:])
    nc.scalar.dma_start(w3t[:], w3t_d.rearrange("p (k c) -> p k c", c=M2P))
    nc.scalar.dma_start(sel_f[:], sel_d[:])
    nc.vector.tensor_copy(sel_r[:], sel_f[:])
    # xg1 lands fully under g0's m-loop
    for q in range(4):
        eng = nc.sync if q % 2 == 0 else nc.scalar
        k0, k1 = 8 * q, min(8 * q + 8, KF_TILES - 1)
        src = xg_d[1].rearrange("(k p) c -> p k c", p=128)
        eng.dma_start(xg1[:, k0:k1, :], src[:, k0:k1, :])
    emit_s1_padding()

    # ---- L2 helpers ----
    # L2 runs in two sequential half-group passes (slots 0,1 then 2,3); each
    # half accumulates its two batch slots in separate PSUM banks (cols 0:368
    # and 512:880 of a [64,1024] tile) so reads never wait on the other slot.
    p3h = {}

    def l2_tile(g, h, from_psum1=False):
        # returns [(tile, col_base)] per local slot
        key = (g, h)
        if key not in p3h:
            if from_psum1:
                p3h[key] = [
                    (psum1.tile([M2P, TW2], f32, tag="ph",
                                name=f"p3_{g}_{h}_{sl}"), 0)
                    for sl in range(2)]
            else:
                t = psum2.tile([M2P, 1024], f32, tag="p3", name=f"p3_{g}_{h}")
                p3h[key] = [(t, 0), (t, 512)]
        return p3h[key]

    def emit_l2_pair(g, h, p, from_psum1=False):
        tiles = l2_tile(g, h, from_psum1)
        for sl in range(2):
            b = 2 * h + sl
            t, c0 = tiles[sl]
            nc.tensor.matmul(
                t[:, c0:c0 + TW2],
                w3t[:, 2 * p:2 * p + 2, :],
                s1g[g][:, 2 * p:2 * p + 2, b * TW2:(b + 1) * TW2],
                start=(p == 0), stop=(p == 3),
                perf_mode=DR,
            )

    def emit_l2_merge(g, h):
        # regroup the 50 stacked shift rows into a [10, 5, .] tile with 5
        # tiny SBUF->SBUF DMAs (SBUF-op APs must start at partition
        # 0/32/64/96), then merge with bf16 adds (DVE 2x_1p fast path).
        tiles = l2_tile(g, h)
        q50 = qpool.tile([M2, 2, TW2], bf16, tag="q50", name=f"q50_{g}_{h}")
        for sl in range(2):
            t, c0 = tiles[sl]
            nc.vector.tensor_copy(q50[:, sl, :], t[:M2, c0:c0 + TW2])
        q5 = qpool.tile([NOUT, NSHIFT, 2, TW2], bf16, tag="q5", name=f"q5_{g}_{h}")
        dma_eng = [nc.scalar, nc.sync, nc.scalar, nc.sync, nc.scalar]
        for s in range(NSHIFT):
            dma_eng[s].dma_start(q5[:, s, :, :], q50[s * NOUT:(s + 1) * NOUT, :, :])
        acc = opool.tile([NOUT, 2 * T], bf16, tag="acc", name=f"acc_{g}_{h}")
        for sl in range(2):
            a = acc[:, sl * T:(sl + 1) * T]
            nc.vector.tensor_add(a, q5[:, 0, sl, 4:4 + T], q5[:, 1, sl, 3:3 + T])
            nc.vector.tensor_add(a, a, q5[:, 2, sl, 2:2 + T])
            nc.vector.tensor_add(a, a, q5[:, 3, sl, 1:1 + T])
            nc.vector.tensor_add(a, a, q5[:, 4, sl, 0:0 + T])
        return acc

    def emit_l2_out(g, h, acc):
        u3 = opool.tile([NOUT, 2 * T], bf16, tag="u3", name=f"u3_{g}_{h}")
        o3 = opool.tile([NOUT, 2 * T], f32, tag="o3", name=f"o3_{g}_{h}")
        for sl in range(2):
            nc.vector.tensor_tensor_scan(
                u3[:, sl * T:(sl + 1) * T], dec2[:],
                acc[:, sl * T:(sl + 1) * T], 0.0, mult, add)
        for sl in range(2):
            nc.vector.tensor_scalar(
                out=o3[:, sl * T:(sl + 1) * T], in0=u3[:, sl * T:(sl + 1) * T],
                scalar1=THETA, scalar2=None, op0=is_ge)
        for sl in range(2):
            nc.sync.dma_start(out_d[GB * g + 2 * h + sl],
                              o3[:, sl * T:(sl + 1) * T])

    def emit_l2_tail_half(g, h, from_psum1):
        # selector-matmul merge on the (idle) PE, one chain per batch slot:
        # copy -> 5 float32r selector matmuls -> psp scan -> threshold -> DMA.
        # Copies alternate DVE/ACT so the two slot chains run in parallel.
        tiles = l2_tile(g, h, from_psum1)
        u3 = opool.tile([NOUT, 2 * T], bf16, tag="u3", name=f"u3_{g}_{h}")
        o3 = opool.tile([NOUT, 2 * T], f32, tag="o3", name=f"o3_{g}_{h}")
        for sl in range(2):
            t, c0 = tiles[sl]
            q50r = qpool.tile([M2, TW2], f32r, tag=f"q50r{sl}",
                              name=f"q50r_{g}_{h}_{sl}")
            if sl == 0:
                nc.vector.tensor_copy(q50r[:], t[:M2, c0:c0 + TW2])
            else:
                nc.scalar.copy(q50r[:], t[:M2, c0:c0 + TW2])
            h3p = psum1.tile([128, 512], f32, tag="ph", name=f"h3p_{g}_{h}_{sl}")
            for s in range(NSHIFT):
                nc.tensor.matmul(
                    h3p[:NOUT, 0:T], sel_r[:, s, :],
                    q50r[:, 4 - s:4 - s + T],
                    start=(s == 0), stop=(s == NSHIFT - 1),
                )
            nc.vector.tensor_tensor_scan(
                u3[:, sl * T:(sl + 1) * T], dec2[:],
                h3p[:NOUT, 0:T], 0.0, mult, add)
            nc.vector.tensor_scalar(
                out=o3[:, sl * T:(sl + 1) * T], in0=u3[:, sl * T:(sl + 1) * T],
                scalar1=THETA, scalar2=None, op0=is_ge)
            nc.sync.dma_start(out_d[GB * g + 2 * h + sl],
                              o3[:, sl * T:(sl + 1) * T])

    # deferred L2 work: items run right before chain (g, m); overflow -> tail
    sched = [[[] for _ in range(M1_TILES)] for _ in range(NG)]
    tail = []

    def defer(g, m, item):
        if m < M1_TILES:
            sched[g][m].append(item)
        elif g + 1 < NG:
            sched[g + 1][m - M1_TILES].append(item)
        else:
            tail.append(item)

    accs = {}

    def run_item(item):
        kind, g, h, arg = item
        if kind == "pair":
            emit_l2_pair(g, h, arg)
        elif kind == "merge":
            accs[(g, h)] = emit_l2_merge(g, h)
        else:
            emit_l2_out(g, h, accs[(g, h)])

    for g in range(NG):
        defer(g, 5, ("pair", g, 0, 0))
        defer(g, 6, ("pair", g, 0, 1))
        if g < NG - 1:
            defer(g, 8, ("pair", g, 0, 2))
            defer(g, 9, ("pair", g, 0, 3))
            defer(g, 9, ("merge", g, 0, None))
            defer(g, 10, ("pair", g, 1, 0))
            defer(g, 10, ("pair", g, 1, 1))
            defer(g, 10, ("out", g, 0, None))
            defer(g, 10, ("pair", g, 1, 2))
            defer(g, 10, ("pair", g, 1, 3))
            defer(g, 11, ("merge", g, 1, None))
            defer(g, 12, ("out", g, 1, None))

    def emit_post(g, m, phs):
        # psp scans per PSUM chunk tile, chained via f32 state handoff (the
        # scan state is f32 internally and u is f32, so this is bit-identical
        # to one long scan); dec is zeroed at slot seams.  One merged 3D-AP
        # threshold covers all 4 batch slots; the very last m-tile keeps
        # per-slot thresholds so the tail's L2 chains start earlier.
        mw = 128 if m < 6 else 32
        u = upool.tile([128, GW], f32, tag="u", name=f"u_{g}_{m}")
        for ci, (c0, c1) in enumerate(CHUNKS):
            nc.vector.tensor_tensor_scan(
                u[:mw, c0:c1], dec[:mw, c0:c1], phs[ci][:mw, 0:c1 - c0],
                0.0 if ci == 0 else u[:mw, c0 - 1:c0], mult, add)
            for b in range(GB):
                if c1 >= b * TF + T > (c0 if ci else 0):
                    if g == NG - 1 and m in (4, 5) and b >= 2:
                        # relu(sign(u-theta)) on ACT == (u >= theta) except
                        # at exact ties; keeps the DVE clear right before
                        # the tail, whose gating thresholds live there
                        sg = opool.tile([128, T], bf16, tag="sgn",
                                        name=f"sgn_{m}_{b}", bufs=2)
                        nc.scalar.activation(
                            sg[:mw, :], u[:mw, b * TF:b * TF + T],
                            mybir.ActivationFunctionType.Sign,
                            bias=thneg[:mw])
                        nc.scalar.activation(
                            s1g[g][:mw, m, b * TW2 + 4:b * TW2 + 4 + T],
                            sg[:mw, :],
                            mybir.ActivationFunctionType.Relu)
                    else:
                        nc.vector.tensor_scalar(
                            out=s1g[g][:mw, m, b * TW2 + 4:b * TW2 + 4 + T],
                            in0=u[:mw, b * TF:b * TF + T],
                            scalar1=THETA, scalar2=None, op0=is_ge)

    def chain_tiles(g, m):
        return [psum1.tile([128, c1 - c0], f32, tag="ph",
                           name=f"ph_{g}_{m}_{ci}")
                for ci, (c0, c1) in enumerate(CHUNKS)]

    def emit_chain_pair(g, m, phs, j):
        lhs = w1f[:, m, 2 * j:2 * j + 2, :] if m < 6 else \
            w1f6[:, 2 * j:2 * j + 2, :]
        mw = 128 if m < 6 else 32
        for ci, (c0, c1) in enumerate(CHUNKS):
            nc.tensor.matmul(
                phs[ci][:mw, 0:c1 - c0], lhs, xgt[g][:, 2 * j:2 * j + 2, c0:c1],
                start=(j == 0), stop=(j == KF_TILES // 2 - 1),
                perf_mode=DR,
            )

    def emit_chain(g, m, phs):
        # pair-major: the three chunk matmuls of a pair share one lhsT, and
        # consecutive same-lhsT matmuls skip the ~213ns weight reload on
        # hardware (chunk-major ordering measured 1.6x slower end-to-end).
        # The very last chain runs chunks A+B for all pairs first, then C:
        # its A/B scans (and the tail's slot-0/1 chains) start ~1.5us
        # earlier, for one chain's worth of exposed C-chunk weight loads.
        if g == NG - 1 and m >= M1_TILES - 2:
            lhs = (lambda j: w1f[:, m, 2 * j:2 * j + 2, :]) if m < 6 else \
                (lambda j: w1f6[:, 2 * j:2 * j + 2, :])
            mw = 128 if m < 6 else 32
            for j in range(KF_TILES // 2):
                for ci in (0, 1):
                    c0, c1 = CHUNKS[ci]
                    nc.tensor.matmul(
                        phs[ci][:mw, 0:c1 - c0], lhs(j),
                        xgt[g][:, 2 * j:2 * j + 2, c0:c1],
                        start=(j == 0), stop=(j == KF_TILES // 2 - 1),
                        perf_mode=DR,
                    )
                if j == 7 and m == M1_TILES - 1:
                    # inject the L2 pairs whose threshold deps (m0-m3) are
                    # long resolved; the pairs needing m4/m5 thresholds go
                    # at the end of the C pass (the DVE clears m5's
                    # postamble roughly then)
                    emit_l2_pair(g, 1, 0, from_psum1=True)
                    emit_l2_pair(g, 1, 1, from_psum1=True)
            c0, c1 = CHUNKS[2]
            for j in range(KF_TILES // 2):
                nc.tensor.matmul(
                    phs[2][:mw, 0:c1 - c0], lhs(j),
                    xgt[g][:, 2 * j:2 * j + 2, c0:c1],
                    start=(j == 0), stop=(j == KF_TILES // 2 - 1),
                    perf_mode=DR,
                )
                if j == 13 and m == M1_TILES - 1:
                    emit_l2_pair(g, 0, 2)
                    emit_l2_pair(g, 1, 2, from_psum1=True)
        else:
            for j in range(KF_TILES // 2):
                emit_chain_pair(g, m, phs, j)

    for g in range(NG):
        if g == 0:
            # k-major over m0/m1 plus m2's first two chunks: the two 3-tile
            # accumulators live in psum1, m2's A+B chunks borrow the (still
            # idle) 2-bank psum2 ring buffer, so 2.7 chains consume each xg0
            # chunk as it lands instead of serializing behind the load.
            for m in range(3):
                for item in sched[g][m]:
                    run_item(item)
            kphs = [chain_tiles(0, m) for m in range(2)]
            m2ab = psum2.tile([128, 1024], f32, tag="p3", name="ph_0_2ab")
            for j in range(KF_TILES // 2):
                for m in range(2):
                    emit_chain_pair(0, m, kphs[m], j)
                for (c0, c1) in ((0, 512), (512, 1024)):
                    nc.tensor.matmul(
                        m2ab[:, c0:c1], w1f[:, 2, 2 * j:2 * j + 2, :],
                        xgt[0][:, 2 * j:2 * j + 2, c0:c1],
                        start=(j == 0), stop=(j == KF_TILES // 2 - 1),
                        perf_mode=DR,
                    )
            for m in range(2):
                emit_post(0, m, kphs[m])
            # m2: chunk C chain (data resident by now) + scans
            phC = psum1.tile([128, 384], f32, tag="ph", name="ph_0_2_2")
            for j in range(KF_TILES // 2):
                nc.tensor.matmul(
                    phC[:, 0:384], w1f[:, 2, 2 * j:2 * j + 2, :],
                    xgt[0][:, 2 * j:2 * j + 2, 1024:1408],
                    start=(j == 0), stop=(j == KF_TILES // 2 - 1),
                    perf_mode=DR,
                )
            u = upool.tile([128, GW], f32, tag="u", name="u_0_2")
            nc.vector.tensor_tensor_scan(
                u[:, 0:512], dec[:, 0:512], m2ab[:, 0:512], 0.0, mult, add)
            nc.vector.tensor_tensor_scan(
                u[:, 512:1024], dec[:, 512:1024], m2ab[:, 512:1024],
                u[:, 511:512], mult, add)
            nc.vector.tensor_tensor_scan(
                u[:, 1024:1408], dec[:, 1024:1408], phC[:, 0:384],
                u[:, 1023:1024], mult, add)
            for b in range(GB):
                nc.vector.tensor_scalar(
                    out=s1g[0][:, 2, b * TW2 + 4:b * TW2 + 4 + T],
                    in0=u[:, b * TF:b * TF + T],
                    scalar1=THETA, scalar2=None, op0=is_ge)
            m_range = range(3, M1_TILES)
        else:
            m_range = range(M1_TILES)
        for m in m_range:
            for item in sched[g][m]:
                run_item(item)
            phs = chain_tiles(g, m)
            emit_chain(g, m, phs)
            emit_post(g, m, phs)

    # ---- tail: last group's L2 ----
    # Four per-slot selector chains, interleaved so every engine stream is
    # in dependency order: PE [H0p3, mms0, H1p3, mms1, mms2, mms3],
    # DVE [copy1, scan0, thr0, scan1, thr1, copy3, scan2, thr2, scan3,
    # thr3], ACT [copy0, copy2].  H0p2/H1p0-2 were injected mid-chain.
    g = NG - 1
    for item in tail:
        run_item(item)
    tiles0 = l2_tile(g, 0)
    tiles1 = l2_tile(g, 1, True)
    u3 = opool.tile([NOUT, GB * T], bf16, tag="u3t", name="u3_tail", bufs=1)
    o3 = opool.tile([NOUT, GB * T], f32, tag="o3t", name="o3_tail", bufs=1)

    def tail_copy(sl):
        t, c0 = (tiles0 if sl < 2 else tiles1)[sl % 2]
        q = qpool.tile([M2, TW2], f32r, tag=f"q50r{sl}", name=f"q50rt_{sl}",
                       bufs=1)
        nc.scalar.copy(q[:], t[:M2, c0:c0 + TW2])
        return q

    def tail_mms(sl, q):
        h3p = psum1.tile([128, 512], f32, tag="ph", name=f"h3pt_{sl}")
        for s in range(NSHIFT):
            nc.tensor.matmul(
                h3p[:NOUT, 0:T], sel_r[:, s, :], q[:, 4 - s:4 - s + T],
                start=(s == 0), stop=(s == NSHIFT - 1))
        return h3p

    def tail_fin(sl, h3p, use_act=False):
        nc.vector.tensor_tensor_scan(
            u3[:, sl * T:(sl + 1) * T], dec2[:], h3p[:NOUT, 0:T],
            0.0, mult, add)
        if use_act:
            # relu(sign(u - theta)) == (u >= theta) except at u == theta
            # exactly; u3 tops out ~3.6 here, so the tie case is unreachable.
            tmp = opool.tile([NOUT, T], f32, tag="sgn", name=f"sgn_{sl}",
                             bufs=1)
            nc.scalar.activation(tmp[:], u3[:, sl * T:(sl + 1) * T],
                                 mybir.ActivationFunctionType.Sign,
                                 bias=thneg[:NOUT])
            nc.scalar.activation(o3[:, sl * T:(sl + 1) * T], tmp[:],
                                 mybir.ActivationFunctionType.Relu)
        else:
            nc.vector.tensor_scalar(
                out=o3[:, sl * T:(sl + 1) * T],
                in0=u3[:, sl * T:(sl + 1) * T],
                scalar1=THETA, scalar2=None, op0=is_ge)
        nc.sync.dma_start(out_d[GB * g + sl], o3[:, sl * T:(sl + 1) * T])

    emit_l2_pair(g, 0, 3)
    q0 = tail_copy(0)
    q1 = tail_copy(1)
    h0 = tail_mms(0, q0)
    emit_l2_pair(g, 1, 3, from_psum1=True)
    h1 = tail_mms(1, q1)
    tail_fin(0, h0)
    q2 = tail_copy(2)
    h2 = tail_mms(2, q2)
    tail_fin(1, h1)
    q3 = tail_copy(3)
    h3 = tail_mms(3, q3)
    tail_fin(2, h2)
    tail_fin(3, h3)


def make_in_maps(spike_input, w1, d1, w3, d3):
    xg, w1f, w1f6, w3t, sel = _prep_host(spike_input, w1, d1, w3, d3)
    in_maps = []
    for c in range(N_CORES):
        in_maps.append({
            "xg": np.ascontiguousarray(xg[c * NG:(c + 1) * NG]),
            "w1f": w1f,
            "w1f6": w1f6,
            "w3t": w3t,
            "sel": sel,
        })
    return in_maps


def kernel(spike_input, w1, d1, w3, d3):
    from concourse import bass_utils

    spike_input = np.asarray(spike_input, dtype=np.float32)
    w1 = np.asarray(w1, dtype=np.float32)
    d1 = np.asarray(d1, dtype=np.float32)
    w3 = np.asarray(w3, dtype=np.float32)
    d3 = np.asarray(d3, dtype=np.float32)

    nc = _build_nc()
    in_maps = make_in_maps(spike_input, w1, d1, w3, d3)
    res = bass_utils.run_bass_kernel_spmd(nc, in_maps, core_ids=list(range(N_CORES)))
    out = np.concatenate([res.results[c]["out"] for c in range(N_CORES)], axis=0)
    return out.astype(np.float32)
